# revision 1
# baseline (speedup 1.0000x reference)
"""Trainium2 Bass kernel for CLIP + CMP loss (nn_CLIPWithCMPLoss).

Full-input contract: kernel(**inputs) takes the complete arrays and returns the
scalar loss. Internally the batch rows are sharded across 8 NeuronCores; each
core owns B/8 = 512 rows of the [B, B] logits matrix (softmax rows are fully
local), emits per-row statistics {softmax partial sums, target prob, masked
denominator}, and the host combines the 8 cores' stats into the scalar loss.

Math (per row i, t = labels[i], esc = exp(logit_scale)):
  L_ij   = esc * <img_i/|img_i|, txt_j/|txt_j|>
  E_ij   = exp(L_ij - esc)            (esc >= max_j L_ij, so E <= 1: stable)
  s_i    = sum_j E_ij                 -> logsumexp = log(s_i) + esc
  Et_i   = E[i, t]                    -> logp target = log(Et_i) - log(s_i)
  Sm_i   = sum_j E_ij * [labels[j] != labels[i]] * [E_ij > Et_i]
  clip   = mean_i (log s_i - log Et_i)
  cmp_i  = [Sm_i > 0] * Et_i / (Sm_i + EPS * s_i)
  loss   = clip + sum_i cmp_i / B
"""

import sys

if "/opt/trn_rl_repo" not in sys.path:
    sys.path.insert(0, "/opt/trn_rl_repo")

import numpy as np

B = 4096
D = 768
E = 512
P = 128
NCORES = 8
SHARD = B // NCORES          # 512 rows per core
RT = SHARD // P              # 4 row-tiles per core
KD = D // P                  # 6 contraction tiles for the encoders
KE = E // P                  # 4 contraction tiles for the logits matmul
HW_ = 2048                   # half-width of a logits row-tile (PSUM half)
NH = B // HW_                # 2 halves
NSTAT = 3 + NH               # per row-tile stats: s half-sums, Et, Sm half-sums
EPS = 1e-10

_CACHE = {}


def _build(gw):
    """Build + compile the per-core Bass program. gw = gather width for the
    one-hot target gather (labels live in [0, gw))."""
    import concourse.tile as tile
    from concourse import bacc, mybir

    f32 = mybir.dt.float32
    AF = mybir.ActivationFunctionType
    OP = mybir.AluOpType

    nc = bacc.Bacc("TRN2", target_bir_lowering=False, debug=False,
                   num_devices=NCORES)

    d_imagesT = nc.dram_tensor("imagesT", [D, SHARD], f32, kind="ExternalInput").ap()
    d_textsT = nc.dram_tensor("textsT", [D, B], f32, kind="ExternalInput").ap()
    d_wimg = nc.dram_tensor("W_img", [D, E], f32, kind="ExternalInput").ap()
    d_wtxt = nc.dram_tensor("W_txt", [D, E], f32, kind="ExternalInput").ap()
    d_labcol = nc.dram_tensor("labcolb", [P, B], f32, kind="ExternalInput").ap()
    d_iota = nc.dram_tensor("iotab", [P, gw], f32, kind="ExternalInput").ap()
    d_labrow = nc.dram_tensor("labrow", [P, RT], f32, kind="ExternalInput").ap()
    d_ls128 = nc.dram_tensor("ls128", [P, 1], f32, kind="ExternalInput").ap()
    d_ls11 = nc.dram_tensor("ls11", [1, 1], f32, kind="ExternalInput").ap()
    d_stats = nc.dram_tensor("stats", [P, RT * NSTAT], f32, kind="ExternalOutput").ap()

    with tile.TileContext(nc) as tc:
        with tc.tile_pool(name="const", bufs=1) as const, \
             tc.tile_pool(name="embs", bufs=1) as embs:

            labcol_sb = const.tile([P, B], f32)
            nc.sync.dma_start(labcol_sb[:], d_labcol)
            iota_sb = const.tile([P, gw], f32)
            nc.sync.dma_start(iota_sb[:], d_iota)
            labrow_sb = const.tile([P, RT], f32)
            nc.sync.dma_start(labrow_sb[:], d_labrow)
            ls128_sb = const.tile([P, 1], f32)
            nc.sync.dma_start(ls128_sb[:], d_ls128)
            ls11_sb = const.tile([1, 1], f32)
            nc.sync.dma_start(ls11_sb[:], d_ls11)
            ones_col = const.tile([P, 1], f32)
            nc.vector.memset(ones_col[:], 1.0)

            # esc = exp(logit_scale); softmax shift uses bias = -esc
            esc = const.tile([P, 1], f32)
            nc.scalar.activation(esc[:], ls128_sb[:], AF.Exp)
            negesc = const.tile([P, 1], f32)
            nc.vector.tensor_scalar_mul(negesc[:], esc[:], -1.0)

            imgnT = embs.tile([P, KE, SHARD], f32)   # normalized img emb^T (lhsT)
            txtnT = embs.tile([P, KE, B], f32)       # normalized txt emb^T (rhs)
            stats_sb = embs.tile([P, RT * NSTAT], f32)

            # ---------------- encoders (transposed layout) ----------------
            # embT = W.T @ X.T lands as [E(part), cols]; column norms via
            # squares + ones-matmul partition reduction; rn = exp(-.5*ln(ss))
            # (+ logit_scale on the image side, folding esc into the scale).
            with tc.tile_pool(name="encin", bufs=1) as encin, \
                 tc.tile_pool(name="xstream", bufs=2) as xstream, \
                 tc.tile_pool(name="encw", bufs=2) as encw, \
                 tc.tile_pool(name="encps", bufs=1, space="PSUM") as encps, \
                 tc.tile_pool(name="ssps", bufs=2, space="PSUM") as ssps:

                wimg_sb = encin.tile([P, KD, E], f32)
                nc.sync.dma_start(wimg_sb[:], d_wimg.rearrange("(ko ki) e -> ki ko e", ki=P))
                wtxt_sb = encin.tile([P, KD, E], f32)
                nc.sync.dma_start(wtxt_sb[:], d_wtxt.rearrange("(ko ki) e -> ki ko e", ki=P))
                imagesT_sb = encin.tile([P, KD, SHARD], f32)
                nc.sync.dma_start(imagesT_sb[:], d_imagesT.rearrange("(ko ki) n -> ki ko n", ki=P))

                def encode(w_sb, x_src, ncols, out_sb, bias):
                    for n in range(ncols // E):
                        x_sb = x_src(n)
                        enc = encps.tile([P, KE, E], f32)
                        sqs = []
                        for m in range(KE):
                            for k in range(KD):
                                nc.tensor.matmul(
                                    enc[:, m, :],
                                    w_sb[:, k, m * P:(m + 1) * P],
                                    x_sb[:, k, :],
                                    start=(k == 0), stop=(k == KD - 1))
                            sq = encw.tile([P, E], f32, tag="sq")
                            nc.scalar.activation(sq[:], enc[:, m, :], AF.Square)
                            sqs.append(sq)
                        ss = ssps.tile([1, E], f32)
                        for m in range(KE):
                            nc.tensor.matmul(ss[:], ones_col[:], sqs[m][:],
                                             start=(m == 0), stop=(m == KE - 1))
                        lnt = encw.tile([1, E], f32, tag="lnt")
                        nc.scalar.activation(lnt[:], ss[:], AF.Ln)
                        rn = encw.tile([1, E], f32, tag="rn")
                        nc.scalar.activation(rn[:], lnt[:], AF.Exp, bias=bias, scale=-0.5)
                        rnb = encw.tile([P, E], f32, tag="rnb")
                        nc.gpsimd.partition_broadcast(rnb[:], rn[:])
                        for m in range(KE):
                            nc.vector.tensor_tensor(
                                out_sb[:, m, n * E:(n + 1) * E],
                                enc[:, m, :], rnb[:], OP.mult)

                encode(wimg_sb, lambda n: imagesT_sb, SHARD, imgnT, ls11_sb[:])

                def txt_src(n):
                    x = xstream.tile([P, KD, E], f32, tag="xs")
                    nc.sync.dma_start(
                        x[:],
                        d_textsT.rearrange("(ko ki) n -> ki ko n", ki=P)[:, :, n * E:(n + 1) * E])
                    return x

                encode(wtxt_sb, txt_src, B, txtnT, 0.0)

            # ---------------- logits + loss stats ----------------
            with tc.tile_pool(name="psL", bufs=2, space="PSUM") as psL, \
                 tc.tile_pool(name="Epool", bufs=2) as Epool, \
                 tc.tile_pool(name="Gpool", bufs=2) as Gpool, \
                 tc.tile_pool(name="g1pool", bufs=2) as g1pool:

                for t in range(RT):
                    base = t * NSTAT
                    et_col = stats_sb[:, base + NH:base + NH + 1]
                    etile = Epool.tile([P, B], f32, tag="E")
                    for h in range(NH):
                        ps = psL.tile([P, HW_], f32)
                        for nn in range(HW_ // E):
                            n = h * (HW_ // E) + nn
                            for k in range(KE):
                                nc.tensor.matmul(
                                    ps[:, nn * E:(nn + 1) * E],
                                    imgnT[:, k, t * P:(t + 1) * P],
                                    txtnT[:, k, n * E:(n + 1) * E],
                                    start=(k == 0), stop=(k == KE - 1))
                        # E = exp(L - esc), accumulate row partial sums
                        nc.scalar.activation(
                            etile[:, h * HW_:(h + 1) * HW_], ps[:], AF.Exp,
                            bias=negesc[:],
                            accum_out=stats_sb[:, base + h:base + h + 1])
                    # Et = E[i, labels[i]] via one-hot: (iota == labrow) * E
                    scr = g1pool.tile([P, gw], f32, tag="scr")
                    nc.vector.scalar_tensor_tensor(
                        scr[:], iota_sb[:], labrow_sb[:, t:t + 1], etile[:, :gw],
                        op0=OP.is_equal, op1=OP.mult, accum_out=et_col)
                    for h in range(NH):
                        eh = etile[:, h * HW_:(h + 1) * HW_]
                        g = Gpool.tile([P, HW_], f32, tag="G")
                        nc.vector.scalar_tensor_tensor(
                            g[:], eh, et_col, eh, op0=OP.is_gt, op1=OP.mult)
                        gm = Gpool.tile([P, HW_], f32, tag="G")
                        nc.vector.scalar_tensor_tensor(
                            gm[:], labcol_sb[:, h * HW_:(h + 1) * HW_],
                            labrow_sb[:, t:t + 1], g[:],
                            op0=OP.not_equal, op1=OP.mult,
                            accum_out=stats_sb[:, base + NH + 1 + h:base + NH + 2 + h])

                nc.sync.dma_start(d_stats, stats_sb[:])

    nc.compile()
    return nc


def _in_maps(images, texts, labels, W_img, W_txt, logit_scale, gw):
    imagesT = np.ascontiguousarray(images.T)
    textsT = np.ascontiguousarray(texts.T)
    lab_f = labels.astype(np.float32)
    labcolb = np.ascontiguousarray(np.broadcast_to(lab_f, (P, B)))
    iotab = np.ascontiguousarray(
        np.broadcast_to(np.arange(gw, dtype=np.float32), (P, gw)))
    ls = np.float32(logit_scale)
    ls128 = np.full((P, 1), ls, np.float32)
    ls11 = np.full((1, 1), ls, np.float32)
    w_img = np.ascontiguousarray(W_img, np.float32)
    w_txt = np.ascontiguousarray(W_txt, np.float32)

    maps = []
    for c in range(NCORES):
        sl = slice(c * SHARD, (c + 1) * SHARD)
        maps.append({
            "imagesT": np.ascontiguousarray(imagesT[:, sl]),
            "textsT": textsT,
            "W_img": w_img,
            "W_txt": w_txt,
            "labcolb": labcolb,
            "iotab": iotab,
            "labrow": np.ascontiguousarray(lab_f[sl].reshape(RT, P).T),
            "ls128": ls128,
            "ls11": ls11,
        })
    return maps


def _assemble(stats_list):
    """Combine the 8 cores' [P, RT*NSTAT] stats into the scalar loss (f64)."""
    clip_sum = 0.0
    cmp_sum = 0.0
    for arr in stats_list:
        a = arr.reshape(P, RT, NSTAT).astype(np.float64)
        s = a[:, :, 0:NH].sum(axis=2)
        et = a[:, :, NH]
        sm = a[:, :, NH + 1:NH + 1 + NH].sum(axis=2)
        clip_sum += float(np.sum(np.log(s) - np.log(et)))
        cmp_sum += float(np.sum(np.where(sm > 0.0, et / (sm + EPS * s), 0.0)))
    return np.float32(clip_sum / B + cmp_sum / B)


def kernel(images, texts, labels, W_img, W_txt, logit_scale):
    from concourse import bass_utils

    images = np.asarray(images, np.float32)
    texts = np.asarray(texts, np.float32)
    labels = np.asarray(labels)
    W_img = np.asarray(W_img, np.float32)
    W_txt = np.asarray(W_txt, np.float32)
    ls = float(np.asarray(logit_scale, np.float32))

    gw = 1024 if int(labels.max()) < 1024 else B
    if gw not in _CACHE:
        _CACHE[gw] = _build(gw)
    nc = _CACHE[gw]

    maps = _in_maps(images, texts, labels, W_img, W_txt, ls, gw)
    res = bass_utils.run_bass_kernel_spmd(nc, maps, core_ids=list(range(NCORES)))
    return _assemble([res.results[c]["stats"] for c in range(NCORES)])


# revision 7
# speedup vs baseline: 1.6422x; 1.6422x over previous
"""Trainium2 Bass kernel for CLIP + CMP loss (nn_CLIPWithCMPLoss).

Full-input contract: kernel(**inputs) takes the complete arrays and returns the
scalar loss. Internally the batch rows are sharded across 8 NeuronCores; each
core owns B/8 = 512 rows of the [B, B] logits matrix (softmax rows are fully
local), emits per-row statistics {softmax partial sums, target prob, masked
denominator}, and the host combines the 8 cores' stats into the scalar loss.
The text encoder is recomputed per core (collectives are unavailable in this
runtime), with all matmuls in fp32r (round-to-nearest-even, 12 mantissa bits
dropped) which streams at full PE rate, 4x faster than 2-pass fp32.

Math (per row i, t = labels[i], esc = exp(logit_scale)):
  L_ij   = esc * <img_i/|img_i|, txt_j/|txt_j|>
  E_ij   = exp(L_ij - esc)            (esc >= max_j L_ij, so E <= 1: stable)
  s_i    = sum_j E_ij                 -> logsumexp = log(s_i) + esc
  Et_i   = E[i, t]                    -> logp target = log(Et_i) - log(s_i)
  Sm_i   = sum_j E_ij * [labels[j] != labels[i]] * [E_ij > Et_i]
  clip   = mean_i (log s_i - log Et_i)
  cmp_i  = [Sm_i > 0] * Et_i / (Sm_i + EPS * s_i)
  loss   = clip + sum_i cmp_i / B
"""

import sys

if "/opt/trn_rl_repo" not in sys.path:
    sys.path.insert(0, "/opt/trn_rl_repo")

import numpy as np

B = 4096
D = 768
E = 512
P = 128
NCORES = 8
SHARD = B // NCORES          # 512 rows per core
RT = SHARD // P              # 4 row-tiles per core
KD = D // P                  # 6 contraction tiles for the encoders
KE = E // P                  # 4 contraction tiles for the logits matmul
HW_ = 2048                   # half-width of a logits row-tile (PSUM half)
NH = B // HW_                # 2 halves
NSTAT = 3 + NH               # per row-tile stats: s half-sums, Et, Sm half-sums
EPS = 1e-10

MM_DT = "f32r"               # matmul operand dtype: "f32" | "f32r"

_CACHE = {}


def _round_f32r(x):
    """Host-side fp32r rounding: RNE dropping the low 12 mantissa bits
    (bit-exact match of the on-device DVE fp32->fp32r cast)."""
    b = np.ascontiguousarray(x, np.float32).view(np.uint32)
    lsb = (b >> 12) & np.uint32(1)
    out = ((b.astype(np.uint64) + 0x7FF + lsb).astype(np.uint32)
           & np.uint32(0xFFFFF000))
    return out.view(np.float32)


def _build(gw, mm_dt=None):
    """Build + compile the per-core Bass program. gw = gather width for the
    one-hot target gather (labels live in [0, gw))."""
    import concourse.tile as tile
    from concourse import bacc, mybir

    if mm_dt is None:
        mm_dt = MM_DT

    f32 = mybir.dt.float32
    fmm = mybir.dt.float32r if mm_dt == "f32r" else f32
    AF = mybir.ActivationFunctionType
    OP = mybir.AluOpType

    nc = bacc.Bacc("TRN2", target_bir_lowering=False, debug=False,
                   num_devices=NCORES)

    d_imagesT = nc.dram_tensor("imagesT", [D, SHARD], fmm, kind="ExternalInput").ap()
    d_textsT = nc.dram_tensor("textsT", [D, B], fmm, kind="ExternalInput").ap()
    d_wimg = nc.dram_tensor("W_img", [D, E], fmm, kind="ExternalInput").ap()
    d_wtxt = nc.dram_tensor("W_txt", [D, E], fmm, kind="ExternalInput").ap()
    d_labcol = nc.dram_tensor("labcolb", [P, B], f32, kind="ExternalInput").ap()
    d_iota = nc.dram_tensor("iotab", [P, gw], f32, kind="ExternalInput").ap()
    d_labrow = nc.dram_tensor("labrow", [P, RT], f32, kind="ExternalInput").ap()
    d_ls128 = nc.dram_tensor("ls128", [P, 1], f32, kind="ExternalInput").ap()
    d_ls11 = nc.dram_tensor("ls11", [1, 1], f32, kind="ExternalInput").ap()
    d_stats = nc.dram_tensor("stats", [P, RT * NSTAT], f32, kind="ExternalOutput").ap()

    with tile.TileContext(nc) as tc:
        with tc.tile_pool(name="const", bufs=1) as const, \
             tc.tile_pool(name="embs", bufs=1) as embs:

            labcol_sb = const.tile([P, B], f32)
            nc.sync.dma_start(labcol_sb[:], d_labcol)
            iota_sb = const.tile([P, gw], f32)
            nc.sync.dma_start(iota_sb[:], d_iota)
            labrow_sb = const.tile([P, RT], f32)
            nc.sync.dma_start(labrow_sb[:], d_labrow)
            ls128_sb = const.tile([P, 1], f32)
            nc.sync.dma_start(ls128_sb[:], d_ls128)
            ls11_sb = const.tile([1, 1], f32)
            nc.sync.dma_start(ls11_sb[:], d_ls11)
            ones_f = const.tile([P, 1], f32)
            nc.vector.memset(ones_f[:], 1.0)
            ones_col = const.tile([P, 1], fmm)
            nc.vector.tensor_copy(ones_col[:], ones_f[:])

            # esc = exp(logit_scale); softmax shift uses bias = -esc
            esc = const.tile([P, 1], f32)
            nc.scalar.activation(esc[:], ls128_sb[:], AF.Exp)
            negesc = const.tile([P, 1], f32)
            nc.vector.tensor_scalar_mul(negesc[:], esc[:], -1.0)

            imgnT = embs.tile([P, KE, SHARD], fmm)   # normalized img emb^T (lhsT)
            txtnT = embs.tile([P, KE, B], fmm)       # normalized txt emb^T (rhs)
            stats_sb = embs.tile([P, RT * NSTAT], f32)

            # ---------------- encoders (transposed layout) ----------------
            # embT = W.T @ X.T lands as [E(part), cols]. PSUM -> SBUF via ACT
            # Copy (single ACT table set); squares + column sumsq via
            # ones-matmul partition reduction; rn = exp(-.5*ln(ss)) (+
            # logit_scale on the image side, folding esc into the scale).
            with tc.tile_pool(name="encin", bufs=1) as encin, \
                 tc.tile_pool(name="xstream", bufs=2) as xstream, \
                 tc.tile_pool(name="encw", bufs=2) as encw, \
                 tc.tile_pool(name="sqp", bufs=3) as sqp, \
                 tc.tile_pool(name="encps", bufs=4, space="PSUM") as encps, \
                 tc.tile_pool(name="ssps", bufs=2, space="PSUM") as ssps:

                wimg_sb = encin.tile([P, KD, E], fmm)
                nc.sync.dma_start(wimg_sb[:], d_wimg.rearrange("(ko ki) e -> ki ko e", ki=P))
                wtxt_sb = encin.tile([P, KD, E], fmm)
                nc.sync.dma_start(wtxt_sb[:], d_wtxt.rearrange("(ko ki) e -> ki ko e", ki=P))
                imagesT_sb = encin.tile([P, KD, SHARD], fmm)
                nc.sync.dma_start(imagesT_sb[:], d_imagesT.rearrange("(ko ki) n -> ki ko n", ki=P))

                def encode(w_sb, x_src, ncols, out_view, bias):
                    for n in range(ncols // E):
                        x_sb = x_src(n)
                        emb = encw.tile([P, KE * E], f32, tag="emb")
                        ss = ssps.tile([1, E], f32)
                        for m in range(KE):
                            enc = encps.tile([P, E], f32, tag="enc")
                            for k in range(KD):
                                nc.tensor.matmul(
                                    enc[:],
                                    w_sb[:, k, m * P:(m + 1) * P],
                                    x_sb[:, k, :],
                                    start=(k == 0), stop=(k == KD - 1))
                            nc.scalar.copy(emb[:, m * E:(m + 1) * E], enc[:])
                            sq = sqp.tile([P, E], fmm, tag="sq")
                            sq_eng = nc.vector if m % 2 == 0 else nc.gpsimd
                            sq_eng.tensor_tensor(
                                sq[:],
                                emb[:, m * E:(m + 1) * E],
                                emb[:, m * E:(m + 1) * E], OP.mult)
                            nc.tensor.matmul(ss[:], ones_col[:], sq[:],
                                             start=(m == 0), stop=(m == KE - 1))
                        lnt = encw.tile([1, E], f32, tag="lnt")
                        nc.scalar.activation(lnt[:], ss[:], AF.Ln)
                        rn = encw.tile([1, E], f32, tag="rn")
                        nc.scalar.activation(rn[:], lnt[:], AF.Exp, bias=bias, scale=-0.5)
                        rnb = encw.tile([P, E], f32, tag="rnb")
                        nc.gpsimd.partition_broadcast(rnb[:], rn[:])
                        for m in range(KE):
                            nc.vector.tensor_tensor(
                                out_view(m, n),
                                emb[:, m * E:(m + 1) * E], rnb[:], OP.mult)

                encode(wimg_sb, lambda n: imagesT_sb, SHARD,
                       lambda m, n: imgnT[:, m, n * E:(n + 1) * E], ls11_sb[:])

                def txt_src(n):
                    x = xstream.tile([P, KD, E], fmm, tag="xs")
                    nc.sync.dma_start(
                        x[:],
                        d_textsT.rearrange("(ko ki) n -> ki ko n", ki=P)[:, :, n * E:(n + 1) * E])
                    return x

                encode(wtxt_sb, txt_src, B,
                       lambda m, n: txtnT[:, m, n * E:(n + 1) * E], 0.0)

            # ---------------- logits + loss stats ----------------
            with tc.tile_pool(name="psL", bufs=2, space="PSUM") as psL, \
                 tc.tile_pool(name="Epool", bufs=2) as Epool, \
                 tc.tile_pool(name="Gpool", bufs=2) as Gpool, \
                 tc.tile_pool(name="g1pool", bufs=2) as g1pool:

                for t in range(RT):
                    base = t * NSTAT
                    et_col = stats_sb[:, base + NH:base + NH + 1]
                    etile = Epool.tile([P, B], f32, tag="E")
                    for h in range(NH):
                        ps = psL.tile([P, HW_], f32)
                        for nn in range(HW_ // E):
                            n = h * (HW_ // E) + nn
                            for k in range(KE):
                                nc.tensor.matmul(
                                    ps[:, nn * E:(nn + 1) * E],
                                    imgnT[:, k, t * P:(t + 1) * P],
                                    txtnT[:, k, n * E:(n + 1) * E],
                                    start=(k == 0), stop=(k == KE - 1))
                        # E = exp(L - esc), accumulate row partial sums
                        nc.scalar.activation(
                            etile[:, h * HW_:(h + 1) * HW_], ps[:], AF.Exp,
                            bias=negesc[:],
                            accum_out=stats_sb[:, base + h:base + h + 1])
                    # Et = E[i, labels[i]] via one-hot: (iota == labrow) * E
                    scr = g1pool.tile([P, gw], f32, tag="scr")
                    nc.vector.scalar_tensor_tensor(
                        scr[:], iota_sb[:], labrow_sb[:, t:t + 1], etile[:, :gw],
                        op0=OP.is_equal, op1=OP.mult, accum_out=et_col)
                    for h in range(NH):
                        eh = etile[:, h * HW_:(h + 1) * HW_]
                        g = Gpool.tile([P, HW_], f32, tag="G")
                        nc.vector.scalar_tensor_tensor(
                            g[:], eh, et_col, eh, op0=OP.is_gt, op1=OP.mult)
                        gm = Gpool.tile([P, HW_], f32, tag="G")
                        nc.vector.scalar_tensor_tensor(
                            gm[:], labcol_sb[:, h * HW_:(h + 1) * HW_],
                            labrow_sb[:, t:t + 1], g[:],
                            op0=OP.not_equal, op1=OP.mult,
                            accum_out=stats_sb[:, base + NH + 1 + h:base + NH + 2 + h])

                nc.sync.dma_start(d_stats, stats_sb[:])

    nc.compile()
    return nc


def _in_maps(images, texts, labels, W_img, W_txt, logit_scale, gw, mm_dt=None):
    if mm_dt is None:
        mm_dt = MM_DT
    rnd = _round_f32r if mm_dt == "f32r" else (lambda x: np.ascontiguousarray(x, np.float32))
    imagesT = rnd(images.T)
    textsT = rnd(texts.T)
    w_img = rnd(W_img)
    w_txt = rnd(W_txt)
    lab_f = labels.astype(np.float32)
    labcolb = np.ascontiguousarray(np.broadcast_to(lab_f, (P, B)))
    iotab = np.ascontiguousarray(
        np.broadcast_to(np.arange(gw, dtype=np.float32), (P, gw)))
    ls = np.float32(logit_scale)
    ls128 = np.full((P, 1), ls, np.float32)
    ls11 = np.full((1, 1), ls, np.float32)

    maps = []
    for c in range(NCORES):
        sl = slice(c * SHARD, (c + 1) * SHARD)
        maps.append({
            "imagesT": np.ascontiguousarray(imagesT[:, sl]),
            "textsT": textsT,
            "W_img": w_img,
            "W_txt": w_txt,
            "labcolb": labcolb,
            "iotab": iotab,
            "labrow": np.ascontiguousarray(lab_f[sl].reshape(RT, P).T),
            "ls128": ls128,
            "ls11": ls11,
        })
    return maps


def _assemble(stats_list):
    """Combine the 8 cores' [P, RT*NSTAT] stats into the scalar loss (f64)."""
    clip_sum = 0.0
    cmp_sum = 0.0
    for arr in stats_list:
        a = arr.reshape(P, RT, NSTAT).astype(np.float64)
        s = a[:, :, 0:NH].sum(axis=2)
        et = a[:, :, NH]
        sm = a[:, :, NH + 1:NH + 1 + NH].sum(axis=2)
        clip_sum += float(np.sum(np.log(s) - np.log(et)))
        cmp_sum += float(np.sum(np.where(sm > 0.0, et / (sm + EPS * s), 0.0)))
    return np.float32(clip_sum / B + cmp_sum / B)


def kernel(images, texts, labels, W_img, W_txt, logit_scale):
    from concourse import bass_utils

    images = np.asarray(images, np.float32)
    texts = np.asarray(texts, np.float32)
    labels = np.asarray(labels)
    W_img = np.asarray(W_img, np.float32)
    W_txt = np.asarray(W_txt, np.float32)
    ls = float(np.asarray(logit_scale, np.float32))

    gw = 1024 if int(labels.max()) < 1024 else B
    if gw not in _CACHE:
        _CACHE[gw] = _build(gw)
    nc = _CACHE[gw]

    maps = _in_maps(images, texts, labels, W_img, W_txt, ls, gw)
    res = bass_utils.run_bass_kernel_spmd(nc, maps, core_ids=list(range(NCORES)))
    return _assemble([res.results[c]["stats"] for c in range(NCORES)])


# revision 10
# speedup vs baseline: 2.3792x; 1.4488x over previous
"""Trainium2 Bass kernel for CLIP + CMP loss (nn_CLIPWithCMPLoss).

Full-input contract: kernel(**inputs) takes the complete arrays and returns the
scalar loss. Internally the batch rows are sharded across 8 NeuronCores; each
core owns B/8 = 512 rows of the [B, B] logits matrix (softmax rows are fully
local), emits per-row statistics {softmax partial sums, target prob, masked
denominator}, and the host combines the 8 cores' stats into the scalar loss.
The text encoder is recomputed per core (collectives are unavailable in this
runtime), with all matmuls in fp32r (round-to-nearest-even, 12 mantissa bits
dropped) which streams at full PE rate, 4x faster than 2-pass fp32.

Math (per row i, t = labels[i], esc = exp(logit_scale)):
  L_ij   = esc * <img_i/|img_i|, txt_j/|txt_j|>
  E_ij   = exp(L_ij - esc)            (esc >= max_j L_ij, so E <= 1: stable)
  s_i    = sum_j E_ij                 -> logsumexp = log(s_i) + esc
  Et_i   = E[i, t]                    -> logp target = log(Et_i) - log(s_i)
  Sm_i   = sum_j E_ij * [labels[j] != labels[i]] * [E_ij > Et_i]
  clip   = mean_i (log s_i - log Et_i)
  cmp_i  = [Sm_i > 0] * Et_i / (Sm_i + EPS * s_i)
  loss   = clip + sum_i cmp_i / B
"""

import sys

if "/opt/trn_rl_repo" not in sys.path:
    sys.path.insert(0, "/opt/trn_rl_repo")

import numpy as np

B = 4096
D = 768
E = 512
P = 128
NCORES = 8
SHARD = B // NCORES          # 512 rows per core
RT = SHARD // P              # 4 row-tiles per core
KD = D // P                  # 6 contraction tiles for the encoders
KE = E // P                  # 4 contraction tiles for the logits matmul
HW_ = 2048                   # half-width of a logits row-tile (PSUM half)
NH = B // HW_                # 2 halves
NSTAT = 3 + NH               # per row-tile stats: s half-sums, Et, Sm half-sums
EPS = 1e-10

MM_DT = "f32r"               # matmul operand dtype: "f32" | "f32r"

_CACHE = {}


def _round_f32r(x):
    """Host-side fp32r rounding: RNE dropping the low 12 mantissa bits
    (bit-exact match of the on-device DVE fp32->fp32r cast)."""
    b = np.ascontiguousarray(x, np.float32).view(np.uint32)
    lsb = (b >> 12) & np.uint32(1)
    out = ((b.astype(np.uint64) + 0x7FF + lsb).astype(np.uint32)
           & np.uint32(0xFFFFF000))
    return out.view(np.float32)


def _build(gw, mm_dt=None):
    """Build + compile the per-core Bass program. gw = gather width for the
    one-hot target gather (labels live in [0, gw))."""
    import concourse.tile as tile
    from concourse import bacc, mybir

    if mm_dt is None:
        mm_dt = MM_DT

    f32 = mybir.dt.float32
    fmm = mybir.dt.float32r if mm_dt == "f32r" else f32
    AF = mybir.ActivationFunctionType
    OP = mybir.AluOpType

    nc = bacc.Bacc("TRN2", target_bir_lowering=False, debug=False,
                   num_devices=NCORES)

    d_imagesT = nc.dram_tensor("imagesT", [D, SHARD], fmm, kind="ExternalInput").ap()
    d_textsT = nc.dram_tensor("textsT", [D, B], fmm, kind="ExternalInput").ap()
    d_wimg = nc.dram_tensor("W_img", [D, E], fmm, kind="ExternalInput").ap()
    d_wtxt = nc.dram_tensor("W_txt", [D, E], fmm, kind="ExternalInput").ap()
    d_labcol = nc.dram_tensor("labcolb", [P, B], f32, kind="ExternalInput").ap()
    d_iota = nc.dram_tensor("iotab", [P, gw], f32, kind="ExternalInput").ap()
    d_labrow = nc.dram_tensor("labrow", [P, RT], f32, kind="ExternalInput").ap()
    d_ls128 = nc.dram_tensor("ls128", [P, 1], f32, kind="ExternalInput").ap()
    d_ls11 = nc.dram_tensor("ls11", [1, 1], f32, kind="ExternalInput").ap()
    d_stats = nc.dram_tensor("stats", [P, RT * NSTAT], f32, kind="ExternalOutput").ap()

    with tile.TileContext(nc) as tc:
        with tc.tile_pool(name="const", bufs=1) as const, \
             tc.tile_pool(name="embs", bufs=1) as embs:

            labcol_sb = const.tile([P, B], f32)
            nc.sync.dma_start(labcol_sb[:], d_labcol)
            iota_sb = const.tile([P, gw], f32)
            nc.sync.dma_start(iota_sb[:], d_iota)
            labrow_sb = const.tile([P, RT], f32)
            nc.sync.dma_start(labrow_sb[:], d_labrow)
            ls128_sb = const.tile([P, 1], f32)
            nc.sync.dma_start(ls128_sb[:], d_ls128)
            ls11_sb = const.tile([1, 1], f32)
            nc.sync.dma_start(ls11_sb[:], d_ls11)
            ones_f = const.tile([P, 1], f32)
            nc.vector.memset(ones_f[:], 1.0)
            ones_col = const.tile([P, 1], fmm)
            nc.vector.tensor_copy(ones_col[:], ones_f[:])

            # esc = exp(logit_scale); softmax shift uses bias = -esc
            esc = const.tile([P, 1], f32)
            nc.scalar.activation(esc[:], ls128_sb[:], AF.Exp)
            negesc = const.tile([P, 1], f32)
            nc.vector.tensor_scalar_mul(negesc[:], esc[:], -1.0)

            imgnT = embs.tile([P, KE, SHARD], fmm)   # normalized img emb^T (lhsT)
            txtnT = embs.tile([P, KE, B], fmm)       # normalized txt emb^T (rhs)
            stats_sb = embs.tile([P, RT * NSTAT], f32)

            # ---------------- encoders (transposed layout) ----------------
            # embT = W.T @ X.T lands as [E(part), cols]. PSUM -> SBUF via ACT
            # Copy (single ACT table set); squares + column sumsq via
            # ones-matmul partition reduction; rn = exp(-.5*ln(ss)) (+
            # logit_scale on the image side, folding esc into the scale).
            with tc.tile_pool(name="encin", bufs=1) as encin, \
                 tc.tile_pool(name="xstream", bufs=2) as xstream, \
                 tc.tile_pool(name="encw", bufs=1) as encw, \
                 tc.tile_pool(name="sqp", bufs=2) as sqp, \
                 tc.tile_pool(name="encps", bufs=4, space="PSUM") as encps, \
                 tc.tile_pool(name="ssps", bufs=2, space="PSUM") as ssps:

                wimg_sb = encin.tile([P, KD, E], fmm)
                nc.sync.dma_start(wimg_sb[:], d_wimg.rearrange("(ko ki) e -> ki ko e", ki=P))
                wtxt_sb = encin.tile([P, KD, E], fmm)
                nc.sync.dma_start(wtxt_sb[:], d_wtxt.rearrange("(ko ki) e -> ki ko e", ki=P))
                imagesT_sb = encin.tile([P, KD, SHARD], fmm)
                nc.sync.dma_start(imagesT_sb[:], d_imagesT.rearrange("(ko ki) n -> ki ko n", ki=P))

                def encode(w_sb, x_src, ncols, out_view, bias):
                    for n in range(ncols // E):
                        x_sb = x_src(n)
                        emb = encw.tile([P, KE * E], f32, tag="emb")
                        ss = ssps.tile([1, E], f32)
                        for m in range(KE):
                            enc = encps.tile([P, E], f32, tag="enc")
                            for k in range(KD):
                                nc.tensor.matmul(
                                    enc[:],
                                    w_sb[:, k, m * P:(m + 1) * P],
                                    x_sb[:, k, :],
                                    start=(k == 0), stop=(k == KD - 1))
                            nc.scalar.copy(emb[:, m * E:(m + 1) * E], enc[:])
                            sq = sqp.tile([P, E], fmm, tag="sq")
                            sq_eng = nc.vector if m % 2 == 0 else nc.gpsimd
                            sq_eng.tensor_tensor(
                                sq[:],
                                emb[:, m * E:(m + 1) * E],
                                emb[:, m * E:(m + 1) * E], OP.mult)
                            nc.tensor.matmul(ss[:], ones_col[:], sq[:],
                                             start=(m == 0), stop=(m == KE - 1))
                        lnt = encw.tile([1, E], f32, tag="lnt")
                        nc.scalar.activation(lnt[:], ss[:], AF.Ln)
                        rn = encw.tile([1, E], f32, tag="rn")
                        nc.scalar.activation(rn[:], lnt[:], AF.Exp, bias=bias, scale=-0.5)
                        rnb = encw.tile([P, E], f32, tag="rnb")
                        nc.gpsimd.partition_broadcast(rnb[:], rn[:])
                        for m in range(KE):
                            nc.vector.tensor_tensor(
                                out_view(m, n),
                                emb[:, m * E:(m + 1) * E], rnb[:], OP.mult)

                encode(wimg_sb, lambda n: imagesT_sb, SHARD,
                       lambda m, n: imgnT[:, m, n * E:(n + 1) * E], ls11_sb[:])

                def txt_src(n):
                    x = xstream.tile([P, KD, E], fmm, tag="xs")
                    nc.sync.dma_start(
                        x[:],
                        d_textsT.rearrange("(ko ki) n -> ki ko n", ki=P)[:, :, n * E:(n + 1) * E])
                    return x

                # Two-pass text encoder: pass 1 keeps the PE dense (raw embs
                # land in txtnT via ACT copies, squares + sumsq-matmuls
                # interleave), pass 2 batches the whole norm chain once, then
                # scales txtnT in place.
                ss_cat = embs.tile([1, B], f32)
                for n in range(B // E):
                    x_sb = txt_src(n)
                    ss = ssps.tile([1, E], f32, tag="ss2")
                    for m in range(KE):
                        enc = encps.tile([P, E], f32, tag="enc")
                        for k in range(KD):
                            nc.tensor.matmul(
                                enc[:],
                                wtxt_sb[:, k, m * P:(m + 1) * P],
                                x_sb[:, k, :],
                                start=(k == 0), stop=(k == KD - 1))
                        nc.scalar.copy(txtnT[:, m, n * E:(n + 1) * E], enc[:])
                        sq = sqp.tile([P, E], fmm, tag="sq")
                        sq_eng = nc.vector if m % 2 == 0 else nc.gpsimd
                        sq_eng.tensor_tensor(
                            sq[:],
                            txtnT[:, m, n * E:(n + 1) * E],
                            txtnT[:, m, n * E:(n + 1) * E], OP.mult)
                        nc.tensor.matmul(ss[:], ones_col[:], sq[:],
                                         start=(m == 0), stop=(m == KE - 1))
                    nc.vector.tensor_copy(ss_cat[:, n * E:(n + 1) * E], ss[:])
                nc.scalar.activation(ss_cat[:], ss_cat[:], AF.Ln)
                nc.scalar.activation(ss_cat[:], ss_cat[:], AF.Exp, bias=0.0, scale=-0.5)
                for n in range(B // E):
                    rnb2 = encw.tile([P, E], f32, tag="rnb2", bufs=2)
                    nc.gpsimd.partition_broadcast(rnb2[:], ss_cat[:, n * E:(n + 1) * E])
                    for m in range(KE):
                        nc.vector.tensor_tensor(
                            txtnT[:, m, n * E:(n + 1) * E],
                            txtnT[:, m, n * E:(n + 1) * E],
                            rnb2[:], OP.mult)

            # ---------------- logits + loss stats ----------------
            with tc.tile_pool(name="psL", bufs=2, space="PSUM") as psL, \
                 tc.tile_pool(name="Epool", bufs=2) as Epool, \
                 tc.tile_pool(name="Gpool", bufs=2) as Gpool, \
                 tc.tile_pool(name="g1pool", bufs=2) as g1pool:

                for t in range(RT):
                    base = t * NSTAT
                    et_col = stats_sb[:, base + NH:base + NH + 1]
                    etile = Epool.tile([P, B], f32, tag="E")
                    for h in range(NH):
                        ps = psL.tile([P, HW_], f32)
                        for nn in range(HW_ // E):
                            n = h * (HW_ // E) + nn
                            for k in range(KE):
                                nc.tensor.matmul(
                                    ps[:, nn * E:(nn + 1) * E],
                                    imgnT[:, k, t * P:(t + 1) * P],
                                    txtnT[:, k, n * E:(n + 1) * E],
                                    start=(k == 0), stop=(k == KE - 1))
                        # E = exp(L - esc), accumulate row partial sums
                        nc.scalar.activation(
                            etile[:, h * HW_:(h + 1) * HW_], ps[:], AF.Exp,
                            bias=negesc[:],
                            accum_out=stats_sb[:, base + h:base + h + 1])
                    # Et = E[i, labels[i]] via one-hot: (iota == labrow) * E
                    scr = g1pool.tile([P, gw], f32, tag="scr")
                    nc.vector.scalar_tensor_tensor(
                        scr[:], iota_sb[:], labrow_sb[:, t:t + 1], etile[:, :gw],
                        op0=OP.is_equal, op1=OP.mult, accum_out=et_col)
                    for h in range(NH):
                        eh = etile[:, h * HW_:(h + 1) * HW_]
                        g = Gpool.tile([P, HW_], f32, tag="G")
                        nc.vector.scalar_tensor_tensor(
                            g[:], eh, et_col, eh, op0=OP.is_gt, op1=OP.mult)
                        gm = Gpool.tile([P, HW_], f32, tag="G")
                        nc.vector.scalar_tensor_tensor(
                            gm[:], labcol_sb[:, h * HW_:(h + 1) * HW_],
                            labrow_sb[:, t:t + 1], g[:],
                            op0=OP.not_equal, op1=OP.mult,
                            accum_out=stats_sb[:, base + NH + 1 + h:base + NH + 2 + h])

                nc.sync.dma_start(d_stats, stats_sb[:])

    nc.compile()
    return nc


def _in_maps(images, texts, labels, W_img, W_txt, logit_scale, gw, mm_dt=None):
    if mm_dt is None:
        mm_dt = MM_DT
    rnd = _round_f32r if mm_dt == "f32r" else (lambda x: np.ascontiguousarray(x, np.float32))
    imagesT = rnd(images.T)
    textsT = rnd(texts.T)
    w_img = rnd(W_img)
    w_txt = rnd(W_txt)
    lab_f = labels.astype(np.float32)
    labcolb = np.ascontiguousarray(np.broadcast_to(lab_f, (P, B)))
    iotab = np.ascontiguousarray(
        np.broadcast_to(np.arange(gw, dtype=np.float32), (P, gw)))
    ls = np.float32(logit_scale)
    ls128 = np.full((P, 1), ls, np.float32)
    ls11 = np.full((1, 1), ls, np.float32)

    maps = []
    for c in range(NCORES):
        sl = slice(c * SHARD, (c + 1) * SHARD)
        maps.append({
            "imagesT": np.ascontiguousarray(imagesT[:, sl]),
            "textsT": textsT,
            "W_img": w_img,
            "W_txt": w_txt,
            "labcolb": labcolb,
            "iotab": iotab,
            "labrow": np.ascontiguousarray(lab_f[sl].reshape(RT, P).T),
            "ls128": ls128,
            "ls11": ls11,
        })
    return maps


def _assemble(stats_list):
    """Combine the 8 cores' [P, RT*NSTAT] stats into the scalar loss (f64)."""
    clip_sum = 0.0
    cmp_sum = 0.0
    for arr in stats_list:
        a = arr.reshape(P, RT, NSTAT).astype(np.float64)
        s = a[:, :, 0:NH].sum(axis=2)
        et = a[:, :, NH]
        sm = a[:, :, NH + 1:NH + 1 + NH].sum(axis=2)
        clip_sum += float(np.sum(np.log(s) - np.log(et)))
        cmp_sum += float(np.sum(np.where(sm > 0.0, et / (sm + EPS * s), 0.0)))
    return np.float32(clip_sum / B + cmp_sum / B)


def kernel(images, texts, labels, W_img, W_txt, logit_scale):
    from concourse import bass_utils

    images = np.asarray(images, np.float32)
    texts = np.asarray(texts, np.float32)
    labels = np.asarray(labels)
    W_img = np.asarray(W_img, np.float32)
    W_txt = np.asarray(W_txt, np.float32)
    ls = float(np.asarray(logit_scale, np.float32))

    gw = 1024 if int(labels.max()) < 1024 else B
    if gw not in _CACHE:
        _CACHE[gw] = _build(gw)
    nc = _CACHE[gw]

    maps = _in_maps(images, texts, labels, W_img, W_txt, ls, gw)
    res = bass_utils.run_bass_kernel_spmd(nc, maps, core_ids=list(range(NCORES)))
    return _assemble([res.results[c]["stats"] for c in range(NCORES)])


# revision 14
# speedup vs baseline: 2.4072x; 1.0118x over previous
"""Trainium2 Bass kernel for CLIP + CMP loss (nn_CLIPWithCMPLoss).

Full-input contract: kernel(**inputs) takes the complete arrays and returns the
scalar loss. Internally the batch rows are sharded across 8 NeuronCores; each
core owns B/8 = 512 rows of the [B, B] logits matrix (softmax rows are fully
local), emits per-row statistics {softmax partial sums, target prob, masked
denominator}, and the host combines the 8 cores' stats into the scalar loss.
The text encoder is recomputed per core (collectives are unavailable in this
runtime), with all matmuls in fp32r (round-to-nearest-even, 12 mantissa bits
dropped) which streams at full PE rate, 4x faster than 2-pass fp32.

Math (per row i, t = labels[i], esc = exp(logit_scale)):
  L_ij   = esc * <img_i/|img_i|, txt_j/|txt_j|>
  E_ij   = exp(L_ij - esc)            (esc >= max_j L_ij, so E <= 1: stable)
  s_i    = sum_j E_ij                 -> logsumexp = log(s_i) + esc
  Et_i   = E[i, t]                    -> logp target = log(Et_i) - log(s_i)
  Sm_i   = sum_j E_ij * [labels[j] != labels[i]] * [E_ij > Et_i]
  clip   = mean_i (log s_i - log Et_i)
  cmp_i  = [Sm_i > 0] * Et_i / (Sm_i + EPS * s_i)
  loss   = clip + sum_i cmp_i / B
"""

import sys

if "/opt/trn_rl_repo" not in sys.path:
    sys.path.insert(0, "/opt/trn_rl_repo")

import numpy as np

B = 4096
D = 768
E = 512
P = 128
NCORES = 8
SHARD = B // NCORES          # 512 rows per core
RT = SHARD // P              # 4 row-tiles per core
KD = D // P                  # 6 contraction tiles for the encoders
KE = E // P                  # 4 contraction tiles for the logits matmul
HW_ = 2048                   # half-width of a logits row-tile (PSUM half)
NH = B // HW_                # 2 halves
NSTAT = 3 + NH               # per row-tile stats: s half-sums, Et, Sm half-sums
EPS = 1e-10

MM_DT = "f32r"               # matmul operand dtype: "f32" | "f32r"

_CACHE = {}


def _round_f32r(x):
    """Host-side fp32r rounding: RNE dropping the low 12 mantissa bits
    (bit-exact match of the on-device DVE fp32->fp32r cast)."""
    b = np.ascontiguousarray(x, np.float32).view(np.uint32)
    lsb = (b >> 12) & np.uint32(1)
    out = ((b.astype(np.uint64) + 0x7FF + lsb).astype(np.uint32)
           & np.uint32(0xFFFFF000))
    return out.view(np.float32)


def _build(gw, mm_dt=None):
    """Build + compile the per-core Bass program. gw = gather width for the
    one-hot target gather (labels live in [0, gw))."""
    import concourse.tile as tile
    from concourse import bacc, mybir

    if mm_dt is None:
        mm_dt = MM_DT

    f32 = mybir.dt.float32
    fmm = mybir.dt.float32r if mm_dt == "f32r" else f32
    AF = mybir.ActivationFunctionType
    OP = mybir.AluOpType

    nc = bacc.Bacc("TRN2", target_bir_lowering=False, debug=False,
                   num_devices=NCORES)

    d_imagesT = nc.dram_tensor("imagesT", [D, SHARD], fmm, kind="ExternalInput").ap()
    d_textsT = nc.dram_tensor("textsT", [D, B], fmm, kind="ExternalInput").ap()
    d_wimg = nc.dram_tensor("W_img", [D, E], fmm, kind="ExternalInput").ap()
    d_wtxt = nc.dram_tensor("W_txt", [D, E], fmm, kind="ExternalInput").ap()
    d_labcol = nc.dram_tensor("labcolb", [P, B], f32, kind="ExternalInput").ap()
    d_iota = nc.dram_tensor("iotab", [P, gw], f32, kind="ExternalInput").ap()
    d_labrow = nc.dram_tensor("labrow", [P, RT], f32, kind="ExternalInput").ap()
    d_ls128 = nc.dram_tensor("ls128", [P, 1], f32, kind="ExternalInput").ap()
    d_ls11 = nc.dram_tensor("ls11", [1, 1], f32, kind="ExternalInput").ap()
    d_stats = nc.dram_tensor("stats", [P, RT * NSTAT], f32, kind="ExternalOutput").ap()

    with tile.TileContext(nc) as tc:
        with tc.tile_pool(name="const", bufs=1) as const, \
             tc.tile_pool(name="embs", bufs=1) as embs:

            labcol_sb = const.tile([P, B], f32)
            iota_sb = const.tile([P, gw], f32)
            labrow_sb = const.tile([P, RT], f32)
            nc.sync.dma_start(labrow_sb[:], d_labrow)
            ls128_sb = const.tile([P, 1], f32)
            nc.sync.dma_start(ls128_sb[:], d_ls128)
            ls11_sb = const.tile([1, 1], f32)
            nc.sync.dma_start(ls11_sb[:], d_ls11)
            ones_f = const.tile([P, 1], f32)
            nc.vector.memset(ones_f[:], 1.0)
            ones_col = const.tile([P, 1], fmm)
            nc.vector.tensor_copy(ones_col[:], ones_f[:])

            # esc = exp(logit_scale); softmax shift uses bias = -esc
            esc = const.tile([P, 1], f32)
            nc.scalar.activation(esc[:], ls128_sb[:], AF.Exp)
            negesc = const.tile([P, 1], f32)
            nc.vector.tensor_scalar_mul(negesc[:], esc[:], -1.0)

            imgnT = embs.tile([P, KE, SHARD], fmm)   # normalized img emb^T (lhsT)
            txtnT = embs.tile([P, KE, B], fmm)       # normalized txt emb^T (rhs)
            stats_sb = embs.tile([P, RT * NSTAT], f32)

            # ---------------- encoders (transposed layout) ----------------
            # embT = W.T @ X.T lands as [E(part), cols]. PSUM -> SBUF via ACT
            # Copy (single ACT table set); squares + column sumsq via
            # ones-matmul partition reduction; rn = exp(-.5*ln(ss)) (+
            # logit_scale on the image side, folding esc into the scale).
            with tc.tile_pool(name="encin", bufs=1) as encin, \
                 tc.tile_pool(name="xstream", bufs=2) as xstream, \
                 tc.tile_pool(name="encw", bufs=1) as encw, \
                 tc.tile_pool(name="sqp", bufs=2) as sqp, \
                 tc.tile_pool(name="encps", bufs=4, space="PSUM") as encps, \
                 tc.tile_pool(name="ssps", bufs=2, space="PSUM") as ssps:

                wimg_sb = encin.tile([P, KD, E], fmm)
                nc.sync.dma_start(wimg_sb[:], d_wimg.rearrange("(ko ki) e -> ki ko e", ki=P))
                wtxt_sb = encin.tile([P, KD, E], fmm)
                nc.sync.dma_start(wtxt_sb[:], d_wtxt.rearrange("(ko ki) e -> ki ko e", ki=P))
                imagesT_sb = encin.tile([P, KD, SHARD], fmm)
                nc.sync.dma_start(imagesT_sb[:], d_imagesT.rearrange("(ko ki) n -> ki ko n", ki=P))
                # loss-phase constants: issued after the encoder inputs so
                # they don't delay the first matmuls
                nc.sync.dma_start(labcol_sb[:], d_labcol)
                nc.sync.dma_start(iota_sb[:], d_iota)

                # Raw image encoder (norm chain deferred to phase C)
                ss_img = embs.tile([1, E], f32)
                ssi = ssps.tile([1, E], f32, tag="ss2")
                for m in range(KE):
                    enc = encps.tile([P, E], f32, tag="enc")
                    for k in range(KD):
                        nc.tensor.matmul(
                            enc[:],
                            wimg_sb[:, k, m * P:(m + 1) * P],
                            imagesT_sb[:, k, :],
                            start=(k == 0), stop=(k == KD - 1))
                    nc.scalar.copy(imgnT[:, m, :], enc[:])
                    sq = sqp.tile([P, E], fmm, tag="sq")
                    sq_eng = nc.vector if m % 2 == 0 else nc.gpsimd
                    sq_eng.tensor_tensor(sq[:], imgnT[:, m, :], imgnT[:, m, :], OP.mult)
                    nc.tensor.matmul(ssi[:], ones_col[:], sq[:],
                                     start=(m == 0), stop=(m == KE - 1))
                nc.vector.tensor_copy(ss_img[:], ssi[:])

                def txt_src(n):
                    x = xstream.tile([P, KD, E], fmm, tag="xs")
                    nc.sync.dma_start(
                        x[:],
                        d_textsT.rearrange("(ko ki) n -> ki ko n", ki=P)[:, :, n * E:(n + 1) * E])
                    return x

                # Two-pass text encoder: pass 1 keeps the PE dense (raw embs
                # land in txtnT via ACT copies, squares + sumsq-matmuls
                # interleave), pass 2 batches the whole norm chain once, then
                # scales txtnT in place.
                ss_cat = embs.tile([1, B], f32)
                for n in range(B // E):
                    x_sb = txt_src(n)
                    ss = ssps.tile([1, E], f32, tag="ss2")
                    for m in range(KE):
                        enc = encps.tile([P, E], f32, tag="enc")
                        for k in range(KD):
                            nc.tensor.matmul(
                                enc[:],
                                wtxt_sb[:, k, m * P:(m + 1) * P],
                                x_sb[:, k, :],
                                start=(k == 0), stop=(k == KD - 1))
                        nc.scalar.copy(txtnT[:, m, n * E:(n + 1) * E], enc[:])
                        sq = sqp.tile([P, E], fmm, tag="sq")
                        sq_eng = nc.vector if m % 2 == 0 else nc.gpsimd
                        sq_eng.tensor_tensor(
                            sq[:],
                            txtnT[:, m, n * E:(n + 1) * E],
                            txtnT[:, m, n * E:(n + 1) * E], OP.mult)
                        nc.tensor.matmul(ss[:], ones_col[:], sq[:],
                                         start=(m == 0), stop=(m == KE - 1))
                    nc.vector.tensor_copy(ss_cat[:, n * E:(n + 1) * E], ss[:])
                # Phase C: batched norm chains (one Ln set load, one Exp set
                # load), then in-place scaling; image first so the first
                # logits row-tile can start as early as possible.
                nc.scalar.activation(ss_cat[:], ss_cat[:], AF.Ln)
                nc.scalar.activation(ss_img[:], ss_img[:], AF.Ln)
                nc.scalar.activation(ss_img[:], ss_img[:], AF.Exp,
                                     bias=ls11_sb[:], scale=-0.5)
                nc.scalar.activation(ss_cat[:], ss_cat[:], AF.Exp, bias=0.0, scale=-0.5)
                rnbi = encw.tile([P, E], f32, tag="rnb2", bufs=2)
                nc.gpsimd.partition_broadcast(rnbi[:], ss_img[:])
                for m in range(KE):
                    nc.vector.tensor_tensor(
                        imgnT[:, m, :], imgnT[:, m, :], rnbi[:], OP.mult)
                for n in range(B // E):
                    rnb2 = encw.tile([P, E], f32, tag="rnb2", bufs=2)
                    nc.gpsimd.partition_broadcast(rnb2[:], ss_cat[:, n * E:(n + 1) * E])
                    for m in range(KE):
                        nc.vector.tensor_tensor(
                            txtnT[:, m, n * E:(n + 1) * E],
                            txtnT[:, m, n * E:(n + 1) * E],
                            rnb2[:], OP.mult)

            # ---------------- logits + loss stats ----------------
            with tc.tile_pool(name="psL", bufs=2, space="PSUM") as psL, \
                 tc.tile_pool(name="Epool", bufs=2) as Epool, \
                 tc.tile_pool(name="Gpool", bufs=2) as Gpool, \
                 tc.tile_pool(name="g1pool", bufs=2) as g1pool:

                for t in range(RT):
                    base = t * NSTAT
                    et_col = stats_sb[:, base + NH:base + NH + 1]
                    etile = Epool.tile([P, B], f32, tag="E")
                    for h in range(NH):
                        ps = psL.tile([P, HW_], f32)
                        for nn in range(HW_ // E):
                            n = h * (HW_ // E) + nn
                            for k in range(KE):
                                nc.tensor.matmul(
                                    ps[:, nn * E:(nn + 1) * E],
                                    imgnT[:, k, t * P:(t + 1) * P],
                                    txtnT[:, k, n * E:(n + 1) * E],
                                    start=(k == 0), stop=(k == KE - 1))
                        # E = exp(L - esc), accumulate row partial sums
                        nc.scalar.activation(
                            etile[:, h * HW_:(h + 1) * HW_], ps[:], AF.Exp,
                            bias=negesc[:],
                            accum_out=stats_sb[:, base + h:base + h + 1])
                    # Et = E[i, labels[i]] via one-hot: (iota == labrow) * E
                    scr = g1pool.tile([P, gw], f32, tag="scr")
                    nc.vector.scalar_tensor_tensor(
                        scr[:], iota_sb[:], labrow_sb[:, t:t + 1], etile[:, :gw],
                        op0=OP.is_equal, op1=OP.mult, accum_out=et_col)
                    for h in range(NH):
                        eh = etile[:, h * HW_:(h + 1) * HW_]
                        g = Gpool.tile([P, HW_], f32, tag="G")
                        nc.vector.scalar_tensor_tensor(
                            g[:], eh, et_col, eh, op0=OP.is_gt, op1=OP.mult)
                        gm = Gpool.tile([P, HW_], f32, tag="G")
                        nc.vector.scalar_tensor_tensor(
                            gm[:], labcol_sb[:, h * HW_:(h + 1) * HW_],
                            labrow_sb[:, t:t + 1], g[:],
                            op0=OP.not_equal, op1=OP.mult,
                            accum_out=stats_sb[:, base + NH + 1 + h:base + NH + 2 + h])

                nc.sync.dma_start(d_stats, stats_sb[:])

    nc.compile()
    return nc


def _in_maps(images, texts, labels, W_img, W_txt, logit_scale, gw, mm_dt=None):
    if mm_dt is None:
        mm_dt = MM_DT
    rnd = _round_f32r if mm_dt == "f32r" else (lambda x: np.ascontiguousarray(x, np.float32))
    imagesT = rnd(images.T)
    textsT = rnd(texts.T)
    w_img = rnd(W_img)
    w_txt = rnd(W_txt)
    lab_f = labels.astype(np.float32)
    labcolb = np.ascontiguousarray(np.broadcast_to(lab_f, (P, B)))
    iotab = np.ascontiguousarray(
        np.broadcast_to(np.arange(gw, dtype=np.float32), (P, gw)))
    ls = np.float32(logit_scale)
    ls128 = np.full((P, 1), ls, np.float32)
    ls11 = np.full((1, 1), ls, np.float32)

    maps = []
    for c in range(NCORES):
        sl = slice(c * SHARD, (c + 1) * SHARD)
        maps.append({
            "imagesT": np.ascontiguousarray(imagesT[:, sl]),
            "textsT": textsT,
            "W_img": w_img,
            "W_txt": w_txt,
            "labcolb": labcolb,
            "iotab": iotab,
            "labrow": np.ascontiguousarray(lab_f[sl].reshape(RT, P).T),
            "ls128": ls128,
            "ls11": ls11,
        })
    return maps


def _assemble(stats_list):
    """Combine the 8 cores' [P, RT*NSTAT] stats into the scalar loss (f64)."""
    clip_sum = 0.0
    cmp_sum = 0.0
    for arr in stats_list:
        a = arr.reshape(P, RT, NSTAT).astype(np.float64)
        s = a[:, :, 0:NH].sum(axis=2)
        et = a[:, :, NH]
        sm = a[:, :, NH + 1:NH + 1 + NH].sum(axis=2)
        clip_sum += float(np.sum(np.log(s) - np.log(et)))
        cmp_sum += float(np.sum(np.where(sm > 0.0, et / (sm + EPS * s), 0.0)))
    return np.float32(clip_sum / B + cmp_sum / B)


def kernel(images, texts, labels, W_img, W_txt, logit_scale):
    from concourse import bass_utils

    images = np.asarray(images, np.float32)
    texts = np.asarray(texts, np.float32)
    labels = np.asarray(labels)
    W_img = np.asarray(W_img, np.float32)
    W_txt = np.asarray(W_txt, np.float32)
    ls = float(np.asarray(logit_scale, np.float32))

    gw = 1024 if int(labels.max()) < 1024 else B
    if gw not in _CACHE:
        _CACHE[gw] = _build(gw)
    nc = _CACHE[gw]

    maps = _in_maps(images, texts, labels, W_img, W_txt, ls, gw)
    res = bass_utils.run_bass_kernel_spmd(nc, maps, core_ids=list(range(NCORES)))
    return _assemble([res.results[c]["stats"] for c in range(NCORES)])


# revision 15
# speedup vs baseline: 2.4421x; 1.0145x over previous
"""Trainium2 Bass kernel for CLIP + CMP loss (nn_CLIPWithCMPLoss).

Full-input contract: kernel(**inputs) takes the complete arrays and returns the
scalar loss. Internally the batch rows are sharded across 8 NeuronCores; each
core owns B/8 = 512 rows of the [B, B] logits matrix (softmax rows are fully
local), emits per-row statistics {softmax partial sums, target prob, masked
denominator}, and the host combines the 8 cores' stats into the scalar loss.
The text encoder is recomputed per core (collectives are unavailable in this
runtime), with all matmuls in fp32r (round-to-nearest-even, 12 mantissa bits
dropped) which streams at full PE rate, 4x faster than 2-pass fp32.

Math (per row i, t = labels[i], esc = exp(logit_scale)):
  L_ij   = esc * <img_i/|img_i|, txt_j/|txt_j|>
  E_ij   = exp(L_ij - esc)            (esc >= max_j L_ij, so E <= 1: stable)
  s_i    = sum_j E_ij                 -> logsumexp = log(s_i) + esc
  Et_i   = E[i, t]                    -> logp target = log(Et_i) - log(s_i)
  Sm_i   = sum_j E_ij * [labels[j] != labels[i]] * [E_ij > Et_i]
  clip   = mean_i (log s_i - log Et_i)
  cmp_i  = [Sm_i > 0] * Et_i / (Sm_i + EPS * s_i)
  loss   = clip + sum_i cmp_i / B
"""

import sys

if "/opt/trn_rl_repo" not in sys.path:
    sys.path.insert(0, "/opt/trn_rl_repo")

import numpy as np

B = 4096
D = 768
E = 512
P = 128
NCORES = 8
SHARD = B // NCORES          # 512 rows per core
RT = SHARD // P              # 4 row-tiles per core
KD = D // P                  # 6 contraction tiles for the encoders
KE = E // P                  # 4 contraction tiles for the logits matmul
HW_ = 2048                   # half-width of a logits row-tile (PSUM half)
NH = B // HW_                # 2 halves
NSTAT = 3 + NH               # per row-tile stats: s half-sums, Et, Sm half-sums
EPS = 1e-10

MM_DT = "f32r"               # matmul operand dtype: "f32" | "f32r"

_CACHE = {}


def _round_f32r(x):
    """Host-side fp32r rounding: RNE dropping the low 12 mantissa bits
    (bit-exact match of the on-device DVE fp32->fp32r cast)."""
    b = np.ascontiguousarray(x, np.float32).view(np.uint32)
    lsb = (b >> 12) & np.uint32(1)
    out = ((b.astype(np.uint64) + 0x7FF + lsb).astype(np.uint32)
           & np.uint32(0xFFFFF000))
    return out.view(np.float32)


def _build(gw, mm_dt=None):
    """Build + compile the per-core Bass program. gw = gather width for the
    one-hot target gather (labels live in [0, gw))."""
    import concourse.tile as tile
    from concourse import bacc, mybir

    if mm_dt is None:
        mm_dt = MM_DT

    f32 = mybir.dt.float32
    fmm = mybir.dt.float32r if mm_dt == "f32r" else f32
    AF = mybir.ActivationFunctionType
    OP = mybir.AluOpType

    nc = bacc.Bacc("TRN2", target_bir_lowering=False, debug=False,
                   num_devices=NCORES)

    d_imagesT = nc.dram_tensor("imagesT", [D, SHARD], fmm, kind="ExternalInput").ap()
    d_textsT = nc.dram_tensor("textsT", [D, B], fmm, kind="ExternalInput").ap()
    d_wimg = nc.dram_tensor("W_img", [D, E], fmm, kind="ExternalInput").ap()
    d_wtxt = nc.dram_tensor("W_txt", [D, E], fmm, kind="ExternalInput").ap()
    d_labcol = nc.dram_tensor("labcolb", [P, B], f32, kind="ExternalInput").ap()
    d_iota = nc.dram_tensor("iotab", [P, gw], f32, kind="ExternalInput").ap()
    d_labrow = nc.dram_tensor("labrow", [P, RT], f32, kind="ExternalInput").ap()
    d_ls128 = nc.dram_tensor("ls128", [P, 1], f32, kind="ExternalInput").ap()
    d_ls11 = nc.dram_tensor("ls11", [1, 1], f32, kind="ExternalInput").ap()
    d_stats = nc.dram_tensor("stats", [P, RT * NSTAT], f32, kind="ExternalOutput").ap()

    with tile.TileContext(nc) as tc:
        with tc.tile_pool(name="const", bufs=1) as const, \
             tc.tile_pool(name="embs", bufs=1) as embs:

            labcol_sb = const.tile([P, B], f32)
            iota_sb = const.tile([P, gw], f32)
            labrow_sb = const.tile([P, RT], f32)
            nc.sync.dma_start(labrow_sb[:], d_labrow)
            ls128_sb = const.tile([P, 1], f32)
            nc.sync.dma_start(ls128_sb[:], d_ls128)
            ls11_sb = const.tile([1, 1], f32)
            nc.sync.dma_start(ls11_sb[:], d_ls11)
            ones_f = const.tile([P, 1], f32)
            nc.vector.memset(ones_f[:], 1.0)
            ones_col = const.tile([P, 1], fmm)
            nc.vector.tensor_copy(ones_col[:], ones_f[:])

            # esc = exp(logit_scale); softmax shift uses bias = -esc
            esc = const.tile([P, 1], f32)
            nc.scalar.activation(esc[:], ls128_sb[:], AF.Exp)
            negesc = const.tile([P, 1], f32)
            nc.vector.tensor_scalar_mul(negesc[:], esc[:], -1.0)

            imgnT = embs.tile([P, KE, SHARD], fmm)   # normalized img emb^T (lhsT)
            txtnT = embs.tile([P, KE, B], fmm)       # normalized txt emb^T (rhs)
            stats_sb = embs.tile([P, RT * NSTAT], f32)

            # ---------------- encoders (transposed layout) ----------------
            # embT = W.T @ X.T lands as [E(part), cols]. PSUM -> SBUF via ACT
            # Copy (single ACT table set); squares + column sumsq via
            # ones-matmul partition reduction; rn = exp(-.5*ln(ss)) (+
            # logit_scale on the image side, folding esc into the scale).
            with tc.tile_pool(name="encin", bufs=1) as encin, \
                 tc.tile_pool(name="xstream", bufs=2) as xstream, \
                 tc.tile_pool(name="encw", bufs=1) as encw, \
                 tc.tile_pool(name="sqp", bufs=2) as sqp, \
                 tc.tile_pool(name="encps", bufs=4, space="PSUM") as encps, \
                 tc.tile_pool(name="ssps", bufs=2, space="PSUM") as ssps:

                h = KD // 2
                wimg_sb = encin.tile([P, KD, E], fmm)
                wi_src = d_wimg.rearrange("(ko ki) e -> ki ko e", ki=P)
                nc.sync.dma_start(wimg_sb[:, :h, :], wi_src[:, :h, :])
                nc.sync.dma_start(wimg_sb[:, h:, :], wi_src[:, h:, :])
                imagesT_sb = encin.tile([P, KD, SHARD], fmm)
                im_src = d_imagesT.rearrange("(ko ki) n -> ki ko n", ki=P)
                nc.sync.dma_start(imagesT_sb[:, :h, :], im_src[:, :h, :])
                nc.sync.dma_start(imagesT_sb[:, h:, :], im_src[:, h:, :])
                wtxt_sb = encin.tile([P, KD, E], fmm)
                wt_src = d_wtxt.rearrange("(ko ki) e -> ki ko e", ki=P)
                nc.sync.dma_start(wtxt_sb[:, :h, :], wt_src[:, :h, :])
                nc.sync.dma_start(wtxt_sb[:, h:, :], wt_src[:, h:, :])
                # loss-phase constants: issued after the encoder inputs so
                # they don't delay the first matmuls
                nc.sync.dma_start(labcol_sb[:, :B // 2], d_labcol[:, :B // 2])
                nc.sync.dma_start(labcol_sb[:, B // 2:], d_labcol[:, B // 2:])
                nc.sync.dma_start(iota_sb[:], d_iota)

                # Raw image encoder (norm chain deferred to phase C)
                ss_img = embs.tile([1, E], f32)
                ssi = ssps.tile([1, E], f32, tag="ss2")
                for m in range(KE):
                    enc = encps.tile([P, E], f32, tag="enc")
                    for k in range(KD):
                        nc.tensor.matmul(
                            enc[:],
                            wimg_sb[:, k, m * P:(m + 1) * P],
                            imagesT_sb[:, k, :],
                            start=(k == 0), stop=(k == KD - 1))
                    nc.scalar.copy(imgnT[:, m, :], enc[:])
                    sq = sqp.tile([P, E], fmm, tag="sq")
                    sq_eng = nc.vector if m % 2 == 0 else nc.gpsimd
                    sq_eng.tensor_tensor(sq[:], imgnT[:, m, :], imgnT[:, m, :], OP.mult)
                    nc.tensor.matmul(ssi[:], ones_col[:], sq[:],
                                     start=(m == 0), stop=(m == KE - 1))
                nc.vector.tensor_copy(ss_img[:], ssi[:])

                def txt_src(n):
                    x = xstream.tile([P, KD, E], fmm, tag="xs")
                    src = d_textsT.rearrange("(ko ki) n -> ki ko n", ki=P)[:, :, n * E:(n + 1) * E]
                    h = KD // 2
                    nc.sync.dma_start(x[:, :h, :], src[:, :h, :])
                    nc.sync.dma_start(x[:, h:, :], src[:, h:, :])
                    return x

                # Two-pass text encoder: pass 1 keeps the PE dense (raw embs
                # land in txtnT via ACT copies, squares + sumsq-matmuls
                # interleave), pass 2 batches the whole norm chain once, then
                # scales txtnT in place.
                ss_cat = embs.tile([1, B], f32)
                for n in range(B // E):
                    x_sb = txt_src(n)
                    ss = ssps.tile([1, E], f32, tag="ss2")
                    for m in range(KE):
                        enc = encps.tile([P, E], f32, tag="enc")
                        for k in range(KD):
                            nc.tensor.matmul(
                                enc[:],
                                wtxt_sb[:, k, m * P:(m + 1) * P],
                                x_sb[:, k, :],
                                start=(k == 0), stop=(k == KD - 1))
                        nc.scalar.copy(txtnT[:, m, n * E:(n + 1) * E], enc[:])
                        sq = sqp.tile([P, E], fmm, tag="sq")
                        sq_eng = nc.vector if m % 2 == 0 else nc.gpsimd
                        sq_eng.tensor_tensor(
                            sq[:],
                            txtnT[:, m, n * E:(n + 1) * E],
                            txtnT[:, m, n * E:(n + 1) * E], OP.mult)
                        nc.tensor.matmul(ss[:], ones_col[:], sq[:],
                                         start=(m == 0), stop=(m == KE - 1))
                    nc.vector.tensor_copy(ss_cat[:, n * E:(n + 1) * E], ss[:])
                # Phase C: batched norm chains (one Ln set load, one Exp set
                # load), then in-place scaling; image first so the first
                # logits row-tile can start as early as possible.
                nc.scalar.activation(ss_cat[:], ss_cat[:], AF.Ln)
                nc.scalar.activation(ss_img[:], ss_img[:], AF.Ln)
                nc.scalar.activation(ss_img[:], ss_img[:], AF.Exp,
                                     bias=ls11_sb[:], scale=-0.5)
                nc.scalar.activation(ss_cat[:], ss_cat[:], AF.Exp, bias=0.0, scale=-0.5)
                rnbi = encw.tile([P, E], f32, tag="rnb2", bufs=2)
                nc.gpsimd.partition_broadcast(rnbi[:], ss_img[:])
                for m in range(KE):
                    nc.vector.tensor_tensor(
                        imgnT[:, m, :], imgnT[:, m, :], rnbi[:], OP.mult)
                for n in range(B // E):
                    rnb2 = encw.tile([P, E], f32, tag="rnb2", bufs=2)
                    nc.gpsimd.partition_broadcast(rnb2[:], ss_cat[:, n * E:(n + 1) * E])
                    for m in range(KE):
                        nc.vector.tensor_tensor(
                            txtnT[:, m, n * E:(n + 1) * E],
                            txtnT[:, m, n * E:(n + 1) * E],
                            rnb2[:], OP.mult)

            # ---------------- logits + loss stats ----------------
            with tc.tile_pool(name="psL", bufs=2, space="PSUM") as psL, \
                 tc.tile_pool(name="Epool", bufs=2) as Epool, \
                 tc.tile_pool(name="Gpool", bufs=2) as Gpool, \
                 tc.tile_pool(name="g1pool", bufs=2) as g1pool:

                for t in range(RT):
                    base = t * NSTAT
                    et_col = stats_sb[:, base + NH:base + NH + 1]
                    etile = Epool.tile([P, B], f32, tag="E")
                    for h in range(NH):
                        ps = psL.tile([P, HW_], f32)
                        for nn in range(HW_ // E):
                            n = h * (HW_ // E) + nn
                            for k in range(KE):
                                nc.tensor.matmul(
                                    ps[:, nn * E:(nn + 1) * E],
                                    imgnT[:, k, t * P:(t + 1) * P],
                                    txtnT[:, k, n * E:(n + 1) * E],
                                    start=(k == 0), stop=(k == KE - 1))
                        # E = exp(L - esc), accumulate row partial sums
                        nc.scalar.activation(
                            etile[:, h * HW_:(h + 1) * HW_], ps[:], AF.Exp,
                            bias=negesc[:],
                            accum_out=stats_sb[:, base + h:base + h + 1])
                    # Et = E[i, labels[i]] via one-hot: (iota == labrow) * E
                    scr = g1pool.tile([P, gw], f32, tag="scr")
                    nc.vector.scalar_tensor_tensor(
                        scr[:], iota_sb[:], labrow_sb[:, t:t + 1], etile[:, :gw],
                        op0=OP.is_equal, op1=OP.mult, accum_out=et_col)
                    for h in range(NH):
                        eh = etile[:, h * HW_:(h + 1) * HW_]
                        g = Gpool.tile([P, HW_], f32, tag="G")
                        nc.vector.scalar_tensor_tensor(
                            g[:], eh, et_col, eh, op0=OP.is_gt, op1=OP.mult)
                        gm = Gpool.tile([P, HW_], f32, tag="G")
                        nc.vector.scalar_tensor_tensor(
                            gm[:], labcol_sb[:, h * HW_:(h + 1) * HW_],
                            labrow_sb[:, t:t + 1], g[:],
                            op0=OP.not_equal, op1=OP.mult,
                            accum_out=stats_sb[:, base + NH + 1 + h:base + NH + 2 + h])

                nc.sync.dma_start(d_stats, stats_sb[:])

    nc.compile()
    return nc


def _in_maps(images, texts, labels, W_img, W_txt, logit_scale, gw, mm_dt=None):
    if mm_dt is None:
        mm_dt = MM_DT
    rnd = _round_f32r if mm_dt == "f32r" else (lambda x: np.ascontiguousarray(x, np.float32))
    imagesT = rnd(images.T)
    textsT = rnd(texts.T)
    w_img = rnd(W_img)
    w_txt = rnd(W_txt)
    lab_f = labels.astype(np.float32)
    labcolb = np.ascontiguousarray(np.broadcast_to(lab_f, (P, B)))
    iotab = np.ascontiguousarray(
        np.broadcast_to(np.arange(gw, dtype=np.float32), (P, gw)))
    ls = np.float32(logit_scale)
    ls128 = np.full((P, 1), ls, np.float32)
    ls11 = np.full((1, 1), ls, np.float32)

    maps = []
    for c in range(NCORES):
        sl = slice(c * SHARD, (c + 1) * SHARD)
        maps.append({
            "imagesT": np.ascontiguousarray(imagesT[:, sl]),
            "textsT": textsT,
            "W_img": w_img,
            "W_txt": w_txt,
            "labcolb": labcolb,
            "iotab": iotab,
            "labrow": np.ascontiguousarray(lab_f[sl].reshape(RT, P).T),
            "ls128": ls128,
            "ls11": ls11,
        })
    return maps


def _assemble(stats_list):
    """Combine the 8 cores' [P, RT*NSTAT] stats into the scalar loss (f64)."""
    clip_sum = 0.0
    cmp_sum = 0.0
    for arr in stats_list:
        a = arr.reshape(P, RT, NSTAT).astype(np.float64)
        s = a[:, :, 0:NH].sum(axis=2)
        et = a[:, :, NH]
        sm = a[:, :, NH + 1:NH + 1 + NH].sum(axis=2)
        clip_sum += float(np.sum(np.log(s) - np.log(et)))
        cmp_sum += float(np.sum(np.where(sm > 0.0, et / (sm + EPS * s), 0.0)))
    return np.float32(clip_sum / B + cmp_sum / B)


def kernel(images, texts, labels, W_img, W_txt, logit_scale):
    from concourse import bass_utils

    images = np.asarray(images, np.float32)
    texts = np.asarray(texts, np.float32)
    labels = np.asarray(labels)
    W_img = np.asarray(W_img, np.float32)
    W_txt = np.asarray(W_txt, np.float32)
    ls = float(np.asarray(logit_scale, np.float32))

    gw = 1024 if int(labels.max()) < 1024 else B
    if gw not in _CACHE:
        _CACHE[gw] = _build(gw)
    nc = _CACHE[gw]

    maps = _in_maps(images, texts, labels, W_img, W_txt, ls, gw)
    res = bass_utils.run_bass_kernel_spmd(nc, maps, core_ids=list(range(NCORES)))
    return _assemble([res.results[c]["stats"] for c in range(NCORES)])
